# revision 17
# baseline (speedup 1.0000x reference)
"""Cross-attention Bass kernel for Trainium2, 8 NeuronCores, head-sharded.

Reference semantics (see problem): q = RMSNorm_head(x@Wq.T+bq), kv = c@Wkv.T+bkv
(k/v interleaved), k = RMSNorm_head(k), out = softmax(q k^T/sqrt(dh)) v,
merged heads -> [b, n, dim].

Sharding: 16 heads over 8 cores (2 heads each). Each core reads full x, c and
its weight slices; writes out[:, :, i*128:(i+1)*128] (its 2 heads are adjacent
in the output feature dim). No collectives.

Per-core pipeline (all fp32 data, float32r matmuls):
  Phase A: for each 512-row chunk of x/c:
    - PE-transpose chunk -> xT/cT tiles [128k, 512seq] in SBUF
    - W-stationary projections -> qT/kT/vT [dim_head_block, seq] in PSUM
    - RMSNorm entirely in T layout: sumsq via ones-matmul (lhsT = per-head
      indicator columns, with 1/gamma^2 folded for k), sqrt+reciprocal on a
      [2, 512] row, gpsimd partition_broadcast, one DVE multiply.
      gamma_q*gamma_k is folded into Wk/bk on the host.
    - V transposed back to natural [m, dh] with a ones column appended
      (softmax denominator rides the AV matmul).
  Phase B: per (batch, 512-col n-chunk): for each of 16 m-tiles:
    S.T[m,nchunk] = kT.T @ qT (two K=64 matmuls, one per head),
    exp(S.T/8) on ACT (PSUM->SBUF), U.T += V'.T @ expS.T.
    Then PE-transpose U.T -> [n,65], divide by the ones-column sum, store.
"""

import sys

sys.path.insert(0, "/opt/trn_rl_repo")

import numpy as np
from contextlib import ExitStack

import concourse.bass as bass
import concourse.tile as tile
from concourse import bacc, mybir
from concourse.bass_utils import run_bass_kernel_spmd
from concourse.masks import make_identity

F32 = mybir.dt.float32
F32R = mybir.dt.float32r

DIM = 1024
H = 16
DH = 64
B = 2
N = 2048
M = 2048
ROWS = B * N            # 4096 flattened rows
NC = 8
HPC = H // NC           # 2 heads per core
EPS = 1.1920928955078125e-07

LAST_EXEC_TIME_NS = None
LAST_RESULTS = None


def r(ap):
    return ap.bitcast(F32R)


def build_bass(dbg=False):
    nc = bacc.Bacc("TRN2", target_bir_lowering=False, debug=False)

    x = nc.dram_tensor("x", [ROWS, DIM], F32R, kind="ExternalInput")
    c = nc.dram_tensor("c", [ROWS, DIM], F32R, kind="ExternalInput")
    wq = nc.dram_tensor("wq", [DIM, 128], F32R, kind="ExternalInput")
    wk = nc.dram_tensor("wk", [DIM, 128], F32R, kind="ExternalInput")
    wv = nc.dram_tensor("wv", [DIM, 128], F32R, kind="ExternalInput")
    bq_d = nc.dram_tensor("bq", [128, 1], F32, kind="ExternalInput")
    bk_d = nc.dram_tensor("bk", [128, 1], F32, kind="ExternalInput")
    bv_d = nc.dram_tensor("bv", [128, 1], F32, kind="ExternalInput")
    gq_d = nc.dram_tensor("gq", [128, 2], F32R, kind="ExternalInput")
    gk_d = nc.dram_tensor("gk", [128, 2], F32R, kind="ExternalInput")
    out = nc.dram_tensor("out", [ROWS, 128], F32, kind="ExternalOutput")
    if dbg:
        qt_d = nc.dram_tensor("qt_dbg", [128, ROWS], F32, kind="ExternalOutput")
        kt_d = nc.dram_tensor("kt_dbg", [128, ROWS], F32, kind="ExternalOutput")
        v2_d = nc.dram_tensor("v2_dbg", [128, (ROWS // 128) * 2 * 128], F32,
                              kind="ExternalOutput")

    NKB = DIM // 128          # 8 k-tiles
    NCHUNK = ROWS // 512      # 8 chunks of 512 rows
    MT_PER_B = N // 128       # 16 m-tiles per batch

    with tile.TileContext(nc) as tc, ExitStack() as ctx:
        const = ctx.enter_context(tc.tile_pool(name="const", bufs=1))
        resid = ctx.enter_context(tc.tile_pool(name="resid", bufs=1))

        ident_f = const.tile([128, 128], F32, tag="identf")
        make_identity(nc, ident_f[:])
        ident = const.tile([128, 128], F32R, tag="ident")
        nc.vector.tensor_copy(ident[:], ident_f[:])

        wq_sb = const.tile([128, NKB, 128], F32R, tag="wq")
        wk_sb = const.tile([128, NKB, 128], F32R, tag="wk")
        wv_sb = const.tile([128, NKB, 128], F32R, tag="wv")
        for kb in range(NKB):
            nc.sync.dma_start(wq_sb[:, kb], wq[kb * 128:(kb + 1) * 128, :])
            nc.sync.dma_start(wk_sb[:, kb], wk[kb * 128:(kb + 1) * 128, :])
            nc.sync.dma_start(wv_sb[:, kb], wv[kb * 128:(kb + 1) * 128, :])
        bq_sb = const.tile([128, 1], F32, tag="bq")
        bk_sb = const.tile([128, 1], F32, tag="bk")
        bv_sb = const.tile([128, 1], F32, tag="bv")
        gq_sb = const.tile([128, 2], F32R, tag="gq")
        gk_sb = const.tile([128, 2], F32R, tag="gk")
        eps_sb = const.tile([128, 1], F32, tag="eps")
        nc.gpsimd.memset(eps_sb[:], EPS)
        nc.sync.dma_start(bq_sb[:], bq_d[:])
        nc.sync.dma_start(bk_sb[:], bk_d[:])
        nc.sync.dma_start(bv_sb[:], bv_d[:])
        nc.sync.dma_start(gq_sb[:], gq_d[:])
        nc.sync.dma_start(gk_sb[:], gk_d[:])

        # Residents: qT/kT (normalized, T layout), V' (natural + ones col).
        qt_sb = resid.tile([128, ROWS], F32R, tag="qt")
        kt_sb = resid.tile([128, ROWS], F32R, tag="kt")
        # V' columns per head: [v(64) | ones(1) | zeros(63)] so the AV
        # matmul yields a full 128-partition U (transposable as 128x128).
        v2_sb = resid.tile([128, ROWS // 128, 2, 128], F32R, tag="v2")
        ones64 = const.tile([128, 64], F32, tag="ones64")
        nc.gpsimd.memset(ones64[:], 1.0)
        nc.vector.tensor_copy(
            v2_sb[:, :, :, 64:65],
            ones64[:].rearrange("p (a b c) -> p a b c", a=ROWS // 128, b=2))
        # expander: expand[x, y] = 1 iff y//64 == x (rb[p] = rinv[p//64])
        expand_f = const.tile([2, 128], F32, tag="expand_f")
        nc.gpsimd.memset(expand_f[:], 0.0)
        nc.gpsimd.affine_select(
            out=expand_f[:], in_=expand_f[:],
            compare_op=mybir.AluOpType.is_ge, fill=1.0,
            base=-64, pattern=[[1, 128]], channel_multiplier=-64)
        nc.gpsimd.affine_select(
            out=expand_f[:], in_=expand_f[:],
            compare_op=mybir.AluOpType.is_ge, fill=0.0,
            base=0, pattern=[[1, 128]], channel_multiplier=-64)
        expand_r = const.tile([2, 128], F32R, tag="expand_r")
        nc.vector.tensor_copy(expand_r[:], expand_f[:])
        z1 = const.tile([128, 1], F32, tag="z1")
        nc.gpsimd.memset(z1[:], 0.0)
        nc.vector.tensor_copy(
            v2_sb[:, :, :, 65:128],
            z1[:].broadcast_to((128, ROWS // 128, 2, 63)))

        # ---------------- Phase A: projections + norms -------------------
        with ExitStack() as actx:
            ld = actx.enter_context(tc.tile_pool(name="ld", bufs=6))
            xtp = actx.enter_context(tc.tile_pool(name="xtp", bufs=2))
            tmp = actx.enter_context(tc.tile_pool(name="tmpA", bufs=2))
            small = actx.enter_context(tc.tile_pool(name="small", bufs=2))
            trps = actx.enter_context(
                tc.tile_pool(name="trps", bufs=2, space="PSUM"))
            projps = actx.enter_context(
                tc.tile_pool(name="projps", bufs=2, space="PSUM"))
            ssps = actx.enter_context(
                tc.tile_pool(name="ssps", bufs=1, space="PSUM"))
            vnps = actx.enter_context(
                tc.tile_pool(name="vnps", bufs=2, space="PSUM"))

            def transpose_chunk(src, ci, tag):
                """Load 4 [128,1024] tiles of src chunk ci, PE-transpose to
                8 [128k, 512seq] SBUF tiles."""
                tiles = []
                for t in range(4):
                    lt = ld.tile([128, DIM], F32R, tag="ld")
                    nc.sync.dma_start(
                        lt[:], src[ci * 512 + t * 128: ci * 512 + (t + 1) * 128, :])
                    tiles.append(lt)
                outs = []
                for kb in range(NKB):
                    ps = trps.tile([128, 512], F32, tag="trps")
                    for t in range(4):
                        nc.tensor.transpose(
                            r(ps[:, t * 128:(t + 1) * 128]),
                            tiles[t][:, kb * 128:(kb + 1) * 128],
                            ident[:])
                    sb = xtp.tile([128, 512], F32R, tag=f"xt{kb}")
                    nc.vector.tensor_copy(sb[:], ps[:])
                    outs.append(sb)
                return outs

            def norm_T(lin_ps, bias_sb, g_sb, dst_ap):
                """RMSNorm in T layout: dst = (lin+bias) * rsqrt(mean+eps),
                per head (partitions 0-63 / 64-127)."""
                s_sb = tmp.tile([128, 512], F32, tag="lin")
                nc.vector.tensor_scalar_add(s_sb[:], lin_ps[:], bias_sb[:])
                sq = tmp.tile([128, 512], F32R, tag="sq")
                nc.vector.tensor_tensor(
                    out=sq[:], in0=s_sb[:], in1=s_sb[:],
                    op=mybir.AluOpType.mult)
                ss = ssps.tile([2, 512], F32, tag="ss")
                nc.tensor.matmul(ss[:], g_sb[:], sq[:])
                rms = small.tile([2, 512], F32, tag="rms")
                nc.scalar.activation(
                    rms[:], ss[:], mybir.ActivationFunctionType.Sqrt,
                    bias=eps_sb[0:2, :], scale=1.0 / DH)
                rinv = small.tile([2, 512], F32R, tag="rinv")
                with nc.allow_low_precision(reason="f32r is fp32-width"):
                    nc.vector.reciprocal(rinv[:], rms[:])
                # broadcast rinv rows 0/64 down their 64 partitions via a
                # K=1 outer-product matmul (ones column x rinv row)
                rb = ssps.tile([128, 512], F32, tag="rb")
                nc.tensor.matmul(rb[:], expand_r[:], rinv[:])
                nc.vector.tensor_tensor(
                    out=dst_ap, in0=s_sb[:], in1=rb[:],
                    op=mybir.AluOpType.mult)

            for ci in range(NCHUNK):
                cols = bass.ts(ci, 512)
                # ---- x -> qT ----
                xt = transpose_chunk(x, ci, "xt")
                q_ps = projps.tile([128, 512], F32, tag="proj")
                for kb in range(NKB):
                    nc.tensor.matmul(q_ps[:], wq_sb[:, kb], xt[kb][:],
                                     start=(kb == 0), stop=(kb == NKB - 1))
                norm_T(q_ps, bq_sb, gq_sb, qt_sb[:, cols])

                # ---- c -> kT, vT ----
                ct = transpose_chunk(c, ci, "ct")
                k_ps = projps.tile([128, 512], F32, tag="proj")
                for kb in range(NKB):
                    nc.tensor.matmul(k_ps[:], wk_sb[:, kb], ct[kb][:],
                                     start=(kb == 0), stop=(kb == NKB - 1))
                norm_T(k_ps, bk_sb, gk_sb, kt_sb[:, cols])

                v_ps = projps.tile([128, 512], F32, tag="proj")
                for kb in range(NKB):
                    nc.tensor.matmul(v_ps[:], wv_sb[:, kb], ct[kb][:],
                                     start=(kb == 0), stop=(kb == NKB - 1))
                v_sb = tmp.tile([128, 512], F32R, tag="vsb")
                nc.vector.tensor_scalar_add(v_sb[:], v_ps[:], bv_sb[:])
                # transpose to natural [m,128] and scatter into V' (cols 0-63
                # of each head block; col 64 is the preset ones column)
                vn = vnps.tile([128, 512], F32, tag="vn")
                for t in range(4):
                    nc.tensor.transpose(
                        r(vn[:, t * 128:(t + 1) * 128]),
                        v_sb[:, t * 128:(t + 1) * 128],
                        ident[:])
                mt0 = ci * 4
                nc.vector.tensor_copy(
                    v2_sb[:, mt0:mt0 + 4, :, 0:64],
                    vn[:].rearrange("p (t h e) -> p t h e", t=4, h=2))

        if dbg:
            nc.sync.dma_start(qt_d[:], qt_sb[:].bitcast(F32))
            nc.sync.dma_start(kt_d[:], kt_sb[:].bitcast(F32))
            nc.sync.dma_start(
                v2_d[:],
                v2_sb[:].bitcast(F32).rearrange("p a b e -> p (a b e)"))

        # ---------------- Phase B: attention ------------------------------
        with ExitStack() as bctx:
            esb = bctx.enter_context(tc.tile_pool(name="esb", bufs=3))
            usb = bctx.enter_context(tc.tile_pool(name="usb", bufs=2))
            osb = bctx.enter_context(tc.tile_pool(name="osb", bufs=3))
            rsb = bctx.enter_context(tc.tile_pool(name="rsb", bufs=4))
            sps = bctx.enter_context(
                tc.tile_pool(name="sps", bufs=2, space="PSUM"))
            ups = bctx.enter_context(
                tc.tile_pool(name="ups", bufs=1, space="PSUM"))
            tps = bctx.enter_context(
                tc.tile_pool(name="tps", bufs=2, space="PSUM"))

            for b in range(B):
                for ch in range(N // 512):
                    n0 = b * N + ch * 512
                    ncols = bass.ds(n0, 512)
                    uA = ups.tile([128, 512], F32, tag="uA")
                    uB = ups.tile([128, 512], F32, tag="uB")
                    for mt in range(MT_PER_B):
                        m0 = b * N + mt * 128
                        mcols = bass.ds(m0, 128)
                        s_ps = sps.tile([128, 1024], F32, tag="s")
                        nc.tensor.matmul(
                            s_ps[:, 0:512],
                            kt_sb[0:64, mcols], qt_sb[0:64, ncols])
                        nc.tensor.matmul(
                            s_ps[:, 512:1024],
                            kt_sb[64:128, mcols], qt_sb[64:128, ncols])
                        e_sb = esb.tile([128, 1024], F32R, tag="e")
                        nc.scalar.activation(
                            e_sb[:], s_ps[:], mybir.ActivationFunctionType.Exp,
                            scale=0.125)
                        mtg = b * MT_PER_B + mt
                        nc.tensor.matmul(
                            uA[:], v2_sb[:, mtg, 0], e_sb[:, 0:512],
                            start=(mt == 0), stop=(mt == MT_PER_B - 1),
                            skip_group_check=True)
                        nc.tensor.matmul(
                            uB[:], v2_sb[:, mtg, 1], e_sb[:, 512:1024],
                            start=(mt == 0), stop=(mt == MT_PER_B - 1),
                            skip_group_check=True)
                    uA_sb = usb.tile([128, 512], F32R, tag="uAs")
                    uB_sb = usb.tile([128, 512], F32R, tag="uBs")
                    nc.vector.tensor_copy(uA_sb[:], uA[:])
                    nc.vector.tensor_copy(uB_sb[:], uB[:])
                    for nt in range(4):
                        t_ps = tps.tile([128, 256], F32, tag="t")
                        nc.tensor.transpose(
                            r(t_ps[:, 0:128]),
                            uA_sb[:, nt * 128:(nt + 1) * 128],
                            ident[:])
                        nc.tensor.transpose(
                            r(t_ps[:, 128:256]),
                            uB_sb[:, nt * 128:(nt + 1) * 128],
                            ident[:])
                        rA = rsb.tile([128, 1], F32, tag="rA")
                        rB = rsb.tile([128, 1], F32, tag="rB")
                        nc.vector.reciprocal(rA[:], t_ps[:, 64:65])
                        nc.vector.reciprocal(rB[:], t_ps[:, 192:193])
                        o_sb = osb.tile([128, 128], F32, tag="o")
                        nc.vector.tensor_scalar_mul(
                            o_sb[:, 0:64], t_ps[:, 0:64], rA[:])
                        nc.vector.tensor_scalar_mul(
                            o_sb[:, 64:128], t_ps[:, 128:192], rB[:])
                        nc.sync.dma_start(
                            out[n0 + nt * 128: n0 + (nt + 1) * 128, :], o_sb[:])

    nc.compile()
    return nc


_CACHED_NC = None


def kernel(x, c, Wq, bq, Wkv, bkv, q_gamma, k_gamma, _trace=False, _dbg=False):
    global LAST_EXEC_TIME_NS, LAST_RESULTS, _CACHED_NC

    x = np.asarray(x, dtype=np.float32)
    c = np.asarray(c, dtype=np.float32)
    Wq = np.asarray(Wq, dtype=np.float32)
    bq = np.asarray(bq, dtype=np.float32)
    Wkv = np.asarray(Wkv, dtype=np.float32)
    bkv = np.asarray(bkv, dtype=np.float32)
    q_gamma = np.asarray(q_gamma, dtype=np.float32)
    k_gamma = np.asarray(k_gamma, dtype=np.float32)

    b, n, _ = x.shape
    x_flat = np.ascontiguousarray(x.reshape(ROWS, DIM))
    c_flat = np.ascontiguousarray(c.reshape(ROWS, DIM))

    g2 = q_gamma * k_gamma                      # [64]
    g2_2 = np.tile(g2, HPC)                     # [128] per-core k rows
    d2 = np.arange(DH)

    in_maps = []
    for i in range(NC):
        h0 = i * HPC
        rows_q = np.concatenate(
            [h * DH + d2 for h in range(h0, h0 + HPC)])          # Wq rows
        k_rows = np.concatenate(
            [h * 2 * DH + 2 * d2 for h in range(h0, h0 + HPC)])  # Wkv k rows
        v_rows = k_rows + 1

        wq_t = np.ascontiguousarray(Wq[rows_q].T)                # [1024,128]
        wk_t = np.ascontiguousarray((Wkv[k_rows] * g2_2[:, None]).T)
        wv_t = np.ascontiguousarray(Wkv[v_rows].T)
        bq_l = np.ascontiguousarray(bq[rows_q].reshape(128, 1))
        bk_l = np.ascontiguousarray((bkv[k_rows] * g2_2).reshape(128, 1))
        bv_l = np.ascontiguousarray(bkv[v_rows].reshape(128, 1))

        # sumsq indicator columns: col h is 1 (q) / 1/g2^2 (k, to undo the
        # folded gamma before the variance) on that head's 64 partitions.
        gq_l = np.zeros((128, 2), dtype=np.float32)
        gk_l = np.zeros((128, 2), dtype=np.float32)
        for h in range(HPC):
            gq_l[h * DH:(h + 1) * DH, h] = 1.0
            gk_l[h * DH:(h + 1) * DH, h] = 1.0 / (g2 * g2)
        in_maps.append({
            "x": x_flat, "c": c_flat,
            "wq": wq_t, "wk": wk_t, "wv": wv_t,
            "bq": bq_l, "bk": bk_l, "bv": bv_l,
            "gq": gq_l, "gk": gk_l,
        })

    if _CACHED_NC is None:
        _CACHED_NC = build_bass(dbg=_dbg)
    nc = _CACHED_NC

    res = run_bass_kernel_spmd(
        nc, in_maps, core_ids=list(range(NC)), trace=_trace)
    LAST_EXEC_TIME_NS = res.exec_time_ns
    LAST_RESULTS = res

    outs = [res.results[i]["out"] for i in range(NC)]
    full = np.concatenate(outs, axis=1)          # [4096, 1024]
    return full.reshape(b, n, DIM)


# revision 19
# speedup vs baseline: 43.5010x; 43.5010x over previous
"""Cross-attention Bass kernel for Trainium2, 8 NeuronCores, head-sharded.

Reference semantics (see problem): q = RMSNorm_head(x@Wq.T+bq), kv = c@Wkv.T+bkv
(k/v interleaved), k = RMSNorm_head(k), out = softmax(q k^T/sqrt(dh)) v,
merged heads -> [b, n, dim].

Sharding: 16 heads over 8 cores (2 heads each). Each core reads full x, c and
its weight slices; writes out[:, :, i*128:(i+1)*128] (its 2 heads are adjacent
in the output feature dim). No collectives.

Per-core pipeline (all fp32 data, float32r matmuls):
  Phase A: for each 512-row chunk of x/c:
    - PE-transpose chunk -> xT/cT tiles [128k, 512seq] in SBUF
    - W-stationary projections -> qT/kT/vT [dim_head_block, seq] in PSUM
    - RMSNorm entirely in T layout: sumsq via ones-matmul (lhsT = per-head
      indicator columns, with 1/gamma^2 folded for k), sqrt+reciprocal on a
      [2, 512] row, gpsimd partition_broadcast, one DVE multiply.
      gamma_q*gamma_k is folded into Wk/bk on the host.
    - V transposed back to natural [m, dh] with a ones column appended
      (softmax denominator rides the AV matmul).
  Phase B: per (batch, 512-col n-chunk): for each of 16 m-tiles:
    S.T[m,nchunk] = kT.T @ qT (two K=64 matmuls, one per head),
    exp(S.T/8) on ACT (PSUM->SBUF), U.T += V'.T @ expS.T.
    Then PE-transpose U.T -> [n,65], divide by the ones-column sum, store.
"""

import sys

sys.path.insert(0, "/opt/trn_rl_repo")

import numpy as np
from contextlib import ExitStack

import concourse.bass as bass
import concourse.tile as tile
from concourse import bacc, mybir
from concourse.bass_utils import run_bass_kernel_spmd
from concourse.masks import make_identity

F32 = mybir.dt.float32
F32R = mybir.dt.float32r

DIM = 1024
H = 16
DH = 64
B = 2
N = 2048
M = 2048
ROWS = B * N            # 4096 flattened rows
NC = 8
HPC = H // NC           # 2 heads per core
EPS = 1.1920928955078125e-07

LAST_EXEC_TIME_NS = None
LAST_RESULTS = None
_LAST_IN_MAPS = None


def r(ap):
    return ap.bitcast(F32R)


def build_bass(dbg=False, reps=1):
    nc = bacc.Bacc("TRN2", target_bir_lowering=False, debug=False)

    x = nc.dram_tensor("x", [ROWS, DIM], F32R, kind="ExternalInput")
    c = nc.dram_tensor("c", [ROWS, DIM], F32R, kind="ExternalInput")
    wq = nc.dram_tensor("wq", [DIM, 128], F32R, kind="ExternalInput")
    wk = nc.dram_tensor("wk", [DIM, 128], F32R, kind="ExternalInput")
    wv = nc.dram_tensor("wv", [DIM, 128], F32R, kind="ExternalInput")
    bq_d = nc.dram_tensor("bq", [128, 1], F32, kind="ExternalInput")
    bk_d = nc.dram_tensor("bk", [128, 1], F32, kind="ExternalInput")
    bv_d = nc.dram_tensor("bv", [128, 1], F32, kind="ExternalInput")
    gq_d = nc.dram_tensor("gq", [128, 2], F32R, kind="ExternalInput")
    gk_d = nc.dram_tensor("gk", [128, 2], F32R, kind="ExternalInput")
    out = nc.dram_tensor("out", [ROWS, 128], F32, kind="ExternalOutput")
    if dbg:
        qt_d = nc.dram_tensor("qt_dbg", [128, ROWS], F32, kind="ExternalOutput")
        kt_d = nc.dram_tensor("kt_dbg", [128, ROWS], F32, kind="ExternalOutput")
        v2_d = nc.dram_tensor("v2_dbg", [128, (ROWS // 128) * 2 * 128], F32,
                              kind="ExternalOutput")

    NKB = DIM // 128          # 8 k-tiles
    NCHUNK = ROWS // 512      # 8 chunks of 512 rows
    MT_PER_B = N // 128       # 16 m-tiles per batch

    with tile.TileContext(nc) as tc, ExitStack() as ctx:
        const = ctx.enter_context(tc.tile_pool(name="const", bufs=1))
        resid = ctx.enter_context(tc.tile_pool(name="resid", bufs=1))

        ident_f = const.tile([128, 128], F32, tag="identf")
        make_identity(nc, ident_f[:])
        ident = const.tile([128, 128], F32R, tag="ident")
        nc.vector.tensor_copy(ident[:], ident_f[:])

        wq_sb = const.tile([128, NKB, 128], F32R, tag="wq")
        wk_sb = const.tile([128, NKB, 128], F32R, tag="wk")
        wv_sb = const.tile([128, NKB, 128], F32R, tag="wv")
        for kb in range(NKB):
            nc.sync.dma_start(wq_sb[:, kb], wq[kb * 128:(kb + 1) * 128, :])
            nc.sync.dma_start(wk_sb[:, kb], wk[kb * 128:(kb + 1) * 128, :])
            nc.sync.dma_start(wv_sb[:, kb], wv[kb * 128:(kb + 1) * 128, :])
        bq_sb = const.tile([128, 1], F32, tag="bq")
        bk_sb = const.tile([128, 1], F32, tag="bk")
        bv_sb = const.tile([128, 1], F32, tag="bv")
        gq_sb = const.tile([128, 2], F32R, tag="gq")
        gk_sb = const.tile([128, 2], F32R, tag="gk")
        eps_sb = const.tile([128, 1], F32, tag="eps")
        nc.gpsimd.memset(eps_sb[:], EPS)
        nc.sync.dma_start(bq_sb[:], bq_d[:])
        nc.sync.dma_start(bk_sb[:], bk_d[:])
        nc.sync.dma_start(bv_sb[:], bv_d[:])
        nc.sync.dma_start(gq_sb[:], gq_d[:])
        nc.sync.dma_start(gk_sb[:], gk_d[:])

        # Residents: qT/kT (normalized, T layout), V' (natural + ones col).
        qt_sb = resid.tile([128, ROWS], F32R, tag="qt")
        kt_sb = resid.tile([128, ROWS], F32R, tag="kt")
        # V' columns per head: [v(64) | ones(1) | zeros(63)] so the AV
        # matmul yields a full 128-partition U (transposable as 128x128).
        v2_sb = resid.tile([128, ROWS // 128, 2, 128], F32R, tag="v2")
        ones64 = const.tile([128, 64], F32, tag="ones64")
        nc.gpsimd.memset(ones64[:], 1.0)
        nc.vector.tensor_copy(
            v2_sb[:, :, :, 64:65],
            ones64[:].rearrange("p (a b c) -> p a b c", a=ROWS // 128, b=2))
        # expander: expand[x, y] = 1 iff y//64 == x (rb[p] = rinv[p//64])
        expand_f = const.tile([2, 128], F32, tag="expand_f")
        nc.gpsimd.memset(expand_f[:], 0.0)
        nc.gpsimd.affine_select(
            out=expand_f[:], in_=expand_f[:],
            compare_op=mybir.AluOpType.is_ge, fill=1.0,
            base=-64, pattern=[[1, 128]], channel_multiplier=-64)
        nc.gpsimd.affine_select(
            out=expand_f[:], in_=expand_f[:],
            compare_op=mybir.AluOpType.is_ge, fill=0.0,
            base=0, pattern=[[1, 128]], channel_multiplier=-64)
        expand_r = const.tile([2, 128], F32R, tag="expand_r")
        nc.vector.tensor_copy(expand_r[:], expand_f[:])
        z1 = const.tile([128, 1], F32, tag="z1")
        nc.gpsimd.memset(z1[:], 0.0)
        nc.vector.tensor_copy(
            v2_sb[:, :, :, 65:128],
            z1[:].broadcast_to((128, ROWS // 128, 2, 63)))

        for _rep in range(reps):
            _phases(nc, tc, locals())

    nc.compile()
    return nc


def _phases(nc, tc, env):
    # unpack closure state
    x = env["x"]; c = env["c"]
    wq_sb = env["wq_sb"]; wk_sb = env["wk_sb"]; wv_sb = env["wv_sb"]
    bq_sb = env["bq_sb"]; bk_sb = env["bk_sb"]; bv_sb = env["bv_sb"]
    gq_sb = env["gq_sb"]; gk_sb = env["gk_sb"]
    eps_sb = env["eps_sb"]; expand_r = env["expand_r"]; ident = env["ident"]
    qt_sb = env["qt_sb"]; kt_sb = env["kt_sb"]; v2_sb = env["v2_sb"]
    out = env["out"]
    dbg = env["dbg"]
    if dbg:
        qt_d = env["qt_d"]; kt_d = env["kt_d"]; v2_d = env["v2_d"]
    NKB = env["NKB"]; NCHUNK = env["NCHUNK"]; MT_PER_B = env["MT_PER_B"]

    if True:
        # ---------------- Phase A: projections + norms -------------------
        with ExitStack() as actx:
            ld = actx.enter_context(tc.tile_pool(name="ld", bufs=6))
            xtp = actx.enter_context(tc.tile_pool(name="xtp", bufs=2))
            tmp = actx.enter_context(tc.tile_pool(name="tmpA", bufs=2))
            small = actx.enter_context(tc.tile_pool(name="small", bufs=2))
            trps = actx.enter_context(
                tc.tile_pool(name="trps", bufs=2, space="PSUM"))
            projps = actx.enter_context(
                tc.tile_pool(name="projps", bufs=2, space="PSUM"))
            ssps = actx.enter_context(
                tc.tile_pool(name="ssps", bufs=1, space="PSUM"))
            vnps = actx.enter_context(
                tc.tile_pool(name="vnps", bufs=2, space="PSUM"))

            def transpose_chunk(src, ci, tag):
                """Load 4 [128,1024] tiles of src chunk ci, PE-transpose to
                8 [128k, 512seq] SBUF tiles."""
                tiles = []
                for t in range(4):
                    lt = ld.tile([128, DIM], F32R, tag="ld")
                    nc.sync.dma_start(
                        lt[:], src[ci * 512 + t * 128: ci * 512 + (t + 1) * 128, :])
                    tiles.append(lt)
                outs = []
                for kb in range(NKB):
                    ps = trps.tile([128, 512], F32, tag="trps")
                    for t in range(4):
                        nc.tensor.transpose(
                            r(ps[:, t * 128:(t + 1) * 128]),
                            tiles[t][:, kb * 128:(kb + 1) * 128],
                            ident[:])
                    sb = xtp.tile([128, 512], F32R, tag=f"xt{kb}")
                    nc.vector.tensor_copy(sb[:], ps[:])
                    outs.append(sb)
                return outs

            def norm_T(lin_ps, bias_sb, g_sb, dst_ap):
                """RMSNorm in T layout: dst = (lin+bias) * rsqrt(mean+eps),
                per head (partitions 0-63 / 64-127)."""
                s_sb = tmp.tile([128, 512], F32, tag="lin")
                nc.vector.tensor_scalar_add(s_sb[:], lin_ps[:], bias_sb[:])
                sq = tmp.tile([128, 512], F32R, tag="sq")
                nc.vector.tensor_tensor(
                    out=sq[:], in0=s_sb[:], in1=s_sb[:],
                    op=mybir.AluOpType.mult)
                ss = ssps.tile([2, 512], F32, tag="ss")
                nc.tensor.matmul(ss[:], g_sb[:], sq[:])
                rms = small.tile([2, 512], F32, tag="rms")
                nc.scalar.activation(
                    rms[:], ss[:], mybir.ActivationFunctionType.Sqrt,
                    bias=eps_sb[0:2, :], scale=1.0 / DH)
                rinv = small.tile([2, 512], F32R, tag="rinv")
                with nc.allow_low_precision(reason="f32r is fp32-width"):
                    nc.vector.reciprocal(rinv[:], rms[:])
                # broadcast rinv rows 0/64 down their 64 partitions via a
                # K=1 outer-product matmul (ones column x rinv row)
                rb = ssps.tile([128, 512], F32, tag="rb")
                nc.tensor.matmul(rb[:], expand_r[:], rinv[:])
                nc.vector.tensor_tensor(
                    out=dst_ap, in0=s_sb[:], in1=rb[:],
                    op=mybir.AluOpType.mult)

            for ci in range(NCHUNK):
                cols = bass.ts(ci, 512)
                # ---- x -> qT ----
                xt = transpose_chunk(x, ci, "xt")
                q_ps = projps.tile([128, 512], F32, tag="proj")
                for kb in range(NKB):
                    nc.tensor.matmul(q_ps[:], wq_sb[:, kb], xt[kb][:],
                                     start=(kb == 0), stop=(kb == NKB - 1))
                norm_T(q_ps, bq_sb, gq_sb, qt_sb[:, cols])

                # ---- c -> kT, vT ----
                ct = transpose_chunk(c, ci, "ct")
                k_ps = projps.tile([128, 512], F32, tag="proj")
                for kb in range(NKB):
                    nc.tensor.matmul(k_ps[:], wk_sb[:, kb], ct[kb][:],
                                     start=(kb == 0), stop=(kb == NKB - 1))
                norm_T(k_ps, bk_sb, gk_sb, kt_sb[:, cols])

                v_ps = projps.tile([128, 512], F32, tag="proj")
                for kb in range(NKB):
                    nc.tensor.matmul(v_ps[:], wv_sb[:, kb], ct[kb][:],
                                     start=(kb == 0), stop=(kb == NKB - 1))
                v_sb = tmp.tile([128, 512], F32R, tag="vsb")
                nc.vector.tensor_scalar_add(v_sb[:], v_ps[:], bv_sb[:])
                # transpose to natural [m,128] and scatter into V' (cols 0-63
                # of each head block; col 64 is the preset ones column)
                vn = vnps.tile([128, 512], F32, tag="vn")
                for t in range(4):
                    nc.tensor.transpose(
                        r(vn[:, t * 128:(t + 1) * 128]),
                        v_sb[:, t * 128:(t + 1) * 128],
                        ident[:])
                mt0 = ci * 4
                nc.vector.tensor_copy(
                    v2_sb[:, mt0:mt0 + 4, :, 0:64],
                    vn[:].rearrange("p (t h e) -> p t h e", t=4, h=2))

        if dbg:
            nc.sync.dma_start(qt_d[:], qt_sb[:].bitcast(F32))
            nc.sync.dma_start(kt_d[:], kt_sb[:].bitcast(F32))
            nc.sync.dma_start(
                v2_d[:],
                v2_sb[:].bitcast(F32).rearrange("p a b e -> p (a b e)"))

        # ---------------- Phase B: attention ------------------------------
        with ExitStack() as bctx:
            esb = bctx.enter_context(tc.tile_pool(name="esb", bufs=3))
            usb = bctx.enter_context(tc.tile_pool(name="usb", bufs=2))
            osb = bctx.enter_context(tc.tile_pool(name="osb", bufs=3))
            rsb = bctx.enter_context(tc.tile_pool(name="rsb", bufs=4))
            sps = bctx.enter_context(
                tc.tile_pool(name="sps", bufs=2, space="PSUM"))
            ups = bctx.enter_context(
                tc.tile_pool(name="ups", bufs=1, space="PSUM"))
            tps = bctx.enter_context(
                tc.tile_pool(name="tps", bufs=2, space="PSUM"))

            for b in range(B):
                for ch in range(N // 512):
                    n0 = b * N + ch * 512
                    ncols = bass.ds(n0, 512)
                    uA = ups.tile([128, 512], F32, tag="uA")
                    uB = ups.tile([128, 512], F32, tag="uB")
                    for mt in range(MT_PER_B):
                        m0 = b * N + mt * 128
                        mcols = bass.ds(m0, 128)
                        s_ps = sps.tile([128, 1024], F32, tag="s")
                        nc.tensor.matmul(
                            s_ps[:, 0:512],
                            kt_sb[0:64, mcols], qt_sb[0:64, ncols])
                        nc.tensor.matmul(
                            s_ps[:, 512:1024],
                            kt_sb[64:128, mcols], qt_sb[64:128, ncols])
                        e_sb = esb.tile([128, 1024], F32R, tag="e")
                        nc.scalar.activation(
                            e_sb[:], s_ps[:], mybir.ActivationFunctionType.Exp,
                            scale=0.125)
                        mtg = b * MT_PER_B + mt
                        nc.tensor.matmul(
                            uA[:], v2_sb[:, mtg, 0], e_sb[:, 0:512],
                            start=(mt == 0), stop=(mt == MT_PER_B - 1),
                            skip_group_check=True)
                        nc.tensor.matmul(
                            uB[:], v2_sb[:, mtg, 1], e_sb[:, 512:1024],
                            start=(mt == 0), stop=(mt == MT_PER_B - 1),
                            skip_group_check=True)
                    uA_sb = usb.tile([128, 512], F32R, tag="uAs")
                    uB_sb = usb.tile([128, 512], F32R, tag="uBs")
                    nc.vector.tensor_copy(uA_sb[:], uA[:])
                    nc.vector.tensor_copy(uB_sb[:], uB[:])
                    for nt in range(4):
                        t_ps = tps.tile([128, 256], F32, tag="t")
                        nc.tensor.transpose(
                            r(t_ps[:, 0:128]),
                            uA_sb[:, nt * 128:(nt + 1) * 128],
                            ident[:])
                        nc.tensor.transpose(
                            r(t_ps[:, 128:256]),
                            uB_sb[:, nt * 128:(nt + 1) * 128],
                            ident[:])
                        rA = rsb.tile([128, 1], F32, tag="rA")
                        rB = rsb.tile([128, 1], F32, tag="rB")
                        nc.vector.reciprocal(rA[:], t_ps[:, 64:65])
                        nc.vector.reciprocal(rB[:], t_ps[:, 192:193])
                        o_sb = osb.tile([128, 128], F32, tag="o")
                        nc.vector.tensor_scalar_mul(
                            o_sb[:, 0:64], t_ps[:, 0:64], rA[:])
                        nc.vector.tensor_scalar_mul(
                            o_sb[:, 64:128], t_ps[:, 128:192], rB[:])
                        nc.sync.dma_start(
                            out[n0 + nt * 128: n0 + (nt + 1) * 128, :], o_sb[:])


_CACHED_NC = None


def kernel(x, c, Wq, bq, Wkv, bkv, q_gamma, k_gamma, _trace=False, _dbg=False):
    global LAST_EXEC_TIME_NS, LAST_RESULTS, _CACHED_NC

    x = np.asarray(x, dtype=np.float32)
    c = np.asarray(c, dtype=np.float32)
    Wq = np.asarray(Wq, dtype=np.float32)
    bq = np.asarray(bq, dtype=np.float32)
    Wkv = np.asarray(Wkv, dtype=np.float32)
    bkv = np.asarray(bkv, dtype=np.float32)
    q_gamma = np.asarray(q_gamma, dtype=np.float32)
    k_gamma = np.asarray(k_gamma, dtype=np.float32)

    b, n, _ = x.shape
    x_flat = np.ascontiguousarray(x.reshape(ROWS, DIM))
    c_flat = np.ascontiguousarray(c.reshape(ROWS, DIM))

    g2 = q_gamma * k_gamma                      # [64]
    g2_2 = np.tile(g2, HPC)                     # [128] per-core k rows
    d2 = np.arange(DH)

    in_maps = []
    for i in range(NC):
        h0 = i * HPC
        rows_q = np.concatenate(
            [h * DH + d2 for h in range(h0, h0 + HPC)])          # Wq rows
        k_rows = np.concatenate(
            [h * 2 * DH + 2 * d2 for h in range(h0, h0 + HPC)])  # Wkv k rows
        v_rows = k_rows + 1

        wq_t = np.ascontiguousarray(Wq[rows_q].T)                # [1024,128]
        wk_t = np.ascontiguousarray((Wkv[k_rows] * g2_2[:, None]).T)
        wv_t = np.ascontiguousarray(Wkv[v_rows].T)
        bq_l = np.ascontiguousarray(bq[rows_q].reshape(128, 1))
        bk_l = np.ascontiguousarray((bkv[k_rows] * g2_2).reshape(128, 1))
        bv_l = np.ascontiguousarray(bkv[v_rows].reshape(128, 1))

        # sumsq indicator columns: col h is 1 (q) / 1/g2^2 (k, to undo the
        # folded gamma before the variance) on that head's 64 partitions.
        gq_l = np.zeros((128, 2), dtype=np.float32)
        gk_l = np.zeros((128, 2), dtype=np.float32)
        for h in range(HPC):
            gq_l[h * DH:(h + 1) * DH, h] = 1.0
            gk_l[h * DH:(h + 1) * DH, h] = 1.0 / (g2 * g2)
        in_maps.append({
            "x": x_flat, "c": c_flat,
            "wq": wq_t, "wk": wk_t, "wv": wv_t,
            "bq": bq_l, "bk": bk_l, "bv": bv_l,
            "gq": gq_l, "gk": gk_l,
        })

    global _LAST_IN_MAPS
    _LAST_IN_MAPS = in_maps
    if _CACHED_NC is None:
        _CACHED_NC = build_bass(dbg=_dbg)
    nc = _CACHED_NC

    res = run_bass_kernel_spmd(
        nc, in_maps, core_ids=list(range(NC)), trace=_trace)
    LAST_EXEC_TIME_NS = res.exec_time_ns
    LAST_RESULTS = res

    outs = [res.results[i]["out"] for i in range(NC)]
    full = np.concatenate(outs, axis=1)          # [4096, 1024]
    return full.reshape(b, n, DIM)
